# revision 29
# baseline (speedup 1.0000x reference)
"""Trainium2 Bass kernel for multi-head attention (B=2, S=2048, D=2048, 16 heads).

Sharding: 8 cores = 2 batch groups (data parallel) x 4 tensor-parallel ranks.
Each core computes QKV + attention for its 4 heads over its batch element.
Per 512-row query chunk the cores exchange their (normalized) attention
outputs O^T with an 8-way AllToAll (one per head-pair half), then each core
contracts the full 2048-dim O rows of the query subtile it owns against the
full wo^T.  This moves ~2x fewer collective bytes than reduce-scattering
partial Y and moves the exchange before the out-projection, shrinking the
kernel tail.  The A2A must span all 8 cores (mesh needs >4), so each core
writes its blocks into both batch-groups' destination slots and picks the
correct source half with rank-conditional DMAs (cc_rank).

Layout:
- All device matmuls contract over the partition dim.  Host pre-transposes:
  xt = x^T, per-head q/k weights as [d, hd] blocks, wv as [d, vcols],
  woT = wo^T.
- Q/K are produced in [hd, s] layout (RoPE pairs permuted even|odd so the
  rotation acts on partition halves); V is produced directly in natural
  [s, hd] layout (stationary = xt tile), so no PE transposes anywhere.
- Scores are computed transposed [k, q]:  exp tiles feed PV directly
  (O^T accumulates in PSUM) and the softmax denominator comes from a
  ones-vector matmul; normalization multiplies O^T by a partition-broadcast
  reciprocal.  Softmax scale is folded into the Exp activation.
"""

import sys
import numpy as np
import ml_dtypes

sys.path.insert(0, "/opt/trn_rl_repo")

B, S, D = 2, 2048, 2048
NH, HD = 16, 128
TP = 4            # tensor-parallel ranks per batch group
HL = NH // TP     # heads per core = 4
NDT = D // 128    # 16 d-tiles
NSC = 4           # 512-col s chunks
NQT = S // 128    # 16
NQC = 4           # 512-row query chunks
SM_SCALE = float(HD) ** -0.5
GROUPS8 = [[0, 1, 2, 3, 4, 5, 6, 7]]
CHUNK_ORDER = [2, 3, 1, 0]

_cache = {}


def _build_graph():
    import concourse.mybir as mybir
    import concourse.tile as tile
    from concourse import bacc

    f32 = mybir.dt.float32
    bf16 = mybir.dt.bfloat16
    AF = mybir.ActivationFunctionType
    OP = mybir.AluOpType

    nc = bacc.Bacc("TRN2", target_bir_lowering=False, debug=False, num_devices=8)

    xt_ext = nc.declare_dram_parameter("xt", [D, S], bf16, isOutput=False)
    wqk_ext = nc.declare_dram_parameter("wqk", [2 * HL * 128, NDT * 128], bf16,
                                        isOutput=False)
    wv_ext = nc.declare_dram_parameter("wv", [D, HL * HD], bf16, isOutput=False)
    c2_ext = nc.declare_dram_parameter("c2", [128, S], f32, isOutput=False)
    s2_ext = nc.declare_dram_parameter("s2", [128, S], f32, isOutput=False)
    maskT_ext = nc.declare_dram_parameter("maskT", [512, 512], f32, isOutput=False)
    woT_ext = nc.declare_dram_parameter("woT", [D, D], bf16, isOutput=False)
    out_ext = nc.declare_dram_parameter("out", [NQC * 128, D], bf16, isOutput=True)

    with tile.TileContext(nc) as tc:
        with tc.tile_pool(name="pers", bufs=1) as pers, \
             tc.tile_pool(name="dram", bufs=1, space="DRAM") as dram:
            qk_bf = [pers.tile([128, S], bf16, tag=f"qk{i}", name=f"qk{i}")
                     for i in range(2 * HL)]            # 0-3 q heads, 4-7 k heads
            v_bf = [pers.tile([128, HL * HD], bf16, tag=f"v{t}", name=f"v{t}")
                    for t in range(NQT)]                # [s-tile, 4*hd]
            ones_col = pers.tile([128, 1], bf16, tag="ones", name="ones")
            nc.vector.memset(ones_col[:], 1.0)

            # ---------------- Phase A: QKV projection + RoPE ----------------
            with tc.tile_pool(name="pha", bufs=1) as pha, \
                 tc.tile_pool(name="rope", bufs=2) as ropep, \
                 tc.tile_pool(name="psA", bufs=3, space="PSUM") as psA:
                wq_sb = [pha.tile([128, NDT * 128], bf16, tag=f"wq{et}",
                                  name=f"wq{et}") for et in range(2 * HL)]
                wv_sb = [pha.tile([128, HL * HD], bf16, tag=f"wv{dt}",
                                  name=f"wv{dt}") for dt in range(NDT)]
                xt_sb = [pha.tile([128, S], bf16, tag=f"xt{dt}", name=f"xt{dt}")
                         for dt in range(NDT)]
                c2_sb = pha.tile([128, S], f32, tag="c2", name="c2")
                s2_sb = pha.tile([128, S], f32, tag="s2", name="s2")

                def dma_xt_chunk(sc):
                    for dt in range(NDT):
                        nc.sync.dma_start(
                            out=xt_sb[dt][:, sc * 512:(sc + 1) * 512],
                            in_=xt_ext[dt * 128:(dt + 1) * 128,
                                       sc * 512:(sc + 1) * 512])

                def dma_tab_chunk(sc):
                    cl = slice(sc * 512, (sc + 1) * 512)
                    nc.sync.dma_start(out=c2_sb[:, cl], in_=c2_ext[:, cl])
                    nc.sync.dma_start(out=s2_sb[:, cl], in_=s2_ext[:, cl])

                # DMA issue order tuned so compute starts after ~2.5 MB
                nc.sync.dma_start(out=wq_sb[0][:], in_=wqk_ext[0:128, :])
                dma_xt_chunk(0)
                for et in range(1, 4):
                    nc.sync.dma_start(out=wq_sb[et][:],
                                      in_=wqk_ext[et * 128:(et + 1) * 128, :])
                dma_tab_chunk(0)
                for et in range(4, 2 * HL):
                    nc.sync.dma_start(out=wq_sb[et][:],
                                      in_=wqk_ext[et * 128:(et + 1) * 128, :])
                for dt in range(NDT):
                    nc.sync.dma_start(out=wv_sb[dt][:],
                                      in_=wv_ext[dt * 128:(dt + 1) * 128, :])
                for sc in range(1, NSC):
                    dma_xt_chunk(sc)
                    dma_tab_chunk(sc)

                for sc in range(NSC):
                    cl = slice(sc * 512, (sc + 1) * 512)
                    for et in range(2 * HL):
                        ps = psA.tile([128, 512], f32, tag="psA",
                                      name=f"psA_{sc}_{et}")
                        for dt in range(NDT):
                            nc.tensor.matmul(
                                ps[:], wq_sb[et][:, dt * 128:(dt + 1) * 128],
                                xt_sb[dt][:, cl],
                                start=(dt == 0), stop=(dt == NDT - 1))
                        # u = [r*c; i*c]; w = [-i*s; r*s] (s2n = [-sin; sin],
                        # cross-partition reads stay on the PSUM operand);
                        # qk = u + w = [r*c - i*s; i*c + r*s]
                        u = ropep.tile([128, 512], f32, tag="t1",
                                       name=f"t1_{sc}_{et}")
                        w = ropep.tile([128, 512], f32, tag="t2",
                                       name=f"t2_{sc}_{et}")
                        nc.vector.tensor_tensor(out=u[:], in0=ps[:],
                                                in1=c2_sb[:, cl], op=OP.mult)
                        nc.vector.tensor_tensor(out=w[0:64, :],
                                                in0=ps[64:128, :],
                                                in1=s2_sb[0:64, cl],
                                                op=OP.mult)
                        nc.vector.tensor_tensor(out=w[64:128, :],
                                                in0=ps[0:64, :],
                                                in1=s2_sb[64:128, cl],
                                                op=OP.mult)
                        nc.vector.tensor_tensor(out=qk_bf[et][:, cl],
                                                in0=u[:], in1=w[:], op=OP.add)
                    for stl in range(4):
                        st = sc * 4 + stl
                        psv = psA.tile([128, 512], f32, tag="psA",
                                       name=f"psV_{st}")
                        for dt in range(NDT):
                            nc.tensor.matmul(
                                psv[:], xt_sb[dt][:, st * 128:(st + 1) * 128],
                                wv_sb[dt][:],
                                start=(dt == 0), stop=(dt == NDT - 1))
                        nc.scalar.copy(v_bf[st][:], psv[:])

            # -------- Phase B: attention + AllToAll + local out-proj --------
            with tc.tile_pool(name="phb", bufs=1) as phb, \
                 tc.tile_pool(name="att", bufs=4) as attp, \
                 tc.tile_pool(name="psS", bufs=2, space="PSUM") as psS, \
                 tc.tile_pool(name="psPV", bufs=2, space="PSUM") as psPV, \
                 tc.tile_pool(name="psD", bufs=2, space="PSUM") as psD, \
                 tc.tile_pool(name="psPR", bufs=2, space="PSUM") as psPR:
                maskT_sb = [phb.tile([128, 512], f32, tag=f"mk{j}",
                                     name=f"mk{j}") for j in range(4)]
                woT_sb = [phb.tile([128, D], bf16, tag=f"wo{k}", name=f"wo{k}")
                          for k in range(NDT)]
                for j in range(4):
                    nc.sync.dma_start(out=maskT_sb[j][:],
                                      in_=maskT_ext[j * 128:(j + 1) * 128, :])
                for k in range(NDT):
                    nc.sync.dma_start(out=woT_sb[k][:],
                                      in_=woT_ext[k * 128:(k + 1) * 128, :])

                rank = nc.sync.cc_rank(replica_groups=GROUPS8)
                is_b0 = rank < 4
                is_b1 = rank >= 4
                a2a_out = {}

                def attention(qc, hp):
                    qcl = slice(qc * 512, (qc + 1) * 512)
                    nkt = qc * 4 + 4
                    o2p = attp.tile([128, 1024], bf16, tag="o2p",
                                    name=f"o2p_{qc}_{hp}", bufs=3)
                    ps_pv = [psPV.tile([128, 512], f32, tag="pv",
                                       name=f"pv_{qc}_{hp}_{i}")
                             for i in range(2)]
                    ps_d = [psD.tile([1, 512], f32, tag="d",
                                     name=f"d_{qc}_{hp}_{i}") for i in range(2)]
                    for kt in range(nkt):
                        for i in range(2):
                            h = 2 * hp + i
                            ps_s = psS.tile([128, 512], f32, tag="s",
                                            name=f"s_{qc}_{hp}_{kt}_{i}")
                            nc.tensor.matmul(
                                ps_s[:], qk_bf[HL + h][:, kt * 128:(kt + 1) * 128],
                                qk_bf[h][:, qcl], start=True, stop=True)
                            if kt >= qc * 4:
                                nc.vector.tensor_tensor(
                                    out=ps_s[:], in0=ps_s[:],
                                    in1=maskT_sb[kt - qc * 4][:], op=OP.add)
                            e_sb = attp.tile([128, 512], bf16, tag="e",
                                             name=f"e_{qc}_{hp}_{kt}_{i}",
                                             bufs=4)
                            nc.scalar.activation(e_sb[:], ps_s[:], AF.Exp,
                                                 scale=SM_SCALE)
                            nc.tensor.matmul(ps_d[i][:], ones_col[:], e_sb[:],
                                             start=(kt == 0),
                                             stop=(kt == nkt - 1))
                            nc.tensor.matmul(
                                ps_pv[i][:], v_bf[kt][:, h * 128:(h + 1) * 128],
                                e_sb[:], start=(kt == 0), stop=(kt == nkt - 1))
                    for i in range(2):
                        d_sb = attp.tile([1, 512], f32, tag="dsb",
                                         name=f"dsb_{qc}_{hp}_{i}")
                        nc.scalar.copy(d_sb[:], ps_d[i][:])
                        db = attp.tile([128, 512], f32, tag="db",
                                       name=f"db_{qc}_{hp}_{i}", bufs=2)
                        nc.gpsimd.partition_broadcast(db[:], d_sb[:])
                        rb = attp.tile([128, 512], f32, tag="rb",
                                       name=f"rb_{qc}_{hp}_{i}", bufs=2)
                        nc.vector.reciprocal_approx_fast(rb[:], db[:])
                        # o2p column layout is g-major (g = r*2 + i: dest
                        # rank r, pair member i) so the staging DMA is 3-dim
                        nc.vector.tensor_tensor(
                            out=o2p[:].rearrange("p (r i f) -> p i r f",
                                                 i=2, f=128)[:, i],
                            in0=ps_pv[i][:].rearrange("p (r f) -> p r f",
                                                      f=128),
                            in1=rb[:].rearrange("p (r f) -> p r f", f=128),
                            op=OP.mult)
                    # stage O^T blocks to DRAM grouped by destination core
                    # (same block for both batch-groups' slots) and exchange
                    a_in = dram.tile([2048, 128], bf16, tag=f"ain_{qc}_{hp}",
                                     name=f"ain_{qc}_{hp}")
                    src = o2p[:].rearrange("p (g f) -> p g f", f=128)
                    for half in range(2):
                        dst = a_in[half * 1024:(half + 1) * 1024, :].rearrange(
                            "(g p) f -> p g f", p=128)
                        nc.sync.dma_start(out=dst, in_=src)
                    a_out = dram.tile([2048, 128], bf16, tag=f"aout_{qc}_{hp}",
                                      name=f"aout_{qc}_{hp}")
                    nc.gpsimd.collective_compute(
                        "AllToAll", OP.bypass, replica_groups=GROUPS8,
                        ins=[a_in[:].opt()], outs=[a_out[:].opt()])
                    a2a_out[(qc, hp)] = a_out

                def outproj(qc):
                    # lhs block m = hp*8 + r*2 + i holds global ocol block
                    # (head) k = r*4 + 2*hp + i
                    lhs = attp.tile([128, D], bf16, tag="lhs",
                                    name=f"lhs_{qc}", bufs=2)
                    for hp in range(2):
                        a_out = a2a_out[(qc, hp)]
                        dst = lhs[:, hp * 1024:(hp + 1) * 1024].rearrange(
                            "p (g f) -> p g f", f=128)
                        for b, cond in ((0, is_b0), (1, is_b1)):
                            src = a_out[b * 1024:(b + 1) * 1024, :].rearrange(
                                "(g p) f -> p g f", p=128)
                            nc.sync.dma_start(out=dst, in_=src, cond=cond)
                    # ec columns run in pairs, hp0 blocks first across the
                    # pair, so the first half of the contraction can run
                    # while the hp1 AllToAll is still in flight
                    for eca, ecb in ((0, 1), (2, 3)):
                        pss = {ec: psPR.tile([128, 512], f32, tag="pr",
                                             name=f"pr_{qc}_{ec}")
                               for ec in (eca, ecb)}
                        for hp in range(2):
                            for ec in (eca, ecb):
                                for n in range(8):
                                    r, i = divmod(n, 2)
                                    m = hp * 8 + n
                                    k = r * HL + 2 * hp + i
                                    nc.tensor.matmul(
                                        pss[ec][:],
                                        lhs[:, m * 128:(m + 1) * 128],
                                        woT_sb[k][:, ec * 512:(ec + 1) * 512],
                                        start=(hp == 0 and n == 0),
                                        stop=(hp == 1 and n == 7))
                        for ec in (eca, ecb):
                            fin = attp.tile([128, 512], bf16, tag="fin",
                                            name=f"fin_{qc}_{ec}", bufs=4)
                            nc.scalar.copy(fin[:], pss[ec][:])
                            nc.sync.dma_start(
                                out=out_ext[qc * 128:(qc + 1) * 128,
                                            ec * 512:(ec + 1) * 512],
                                in_=fin[:])

                prev = None
                for qc in CHUNK_ORDER:
                    attention(qc, 0)
                    if prev is not None:
                        outproj(prev)
                    attention(qc, 1)
                    prev = qc
                outproj(prev)
    nc.finalize()
    return nc


def _prep_inputs(x, freqs_cos, freqs_sin, mask, wqkv, wo):
    bf = ml_dtypes.bfloat16
    perm = np.concatenate([np.arange(0, HD, 2), np.arange(1, HD, 2)])
    mask2d = np.asarray(mask, np.float32).reshape(S, S)
    maskT = np.ascontiguousarray(np.concatenate(
        [np.maximum(mask2d[0:512, j * 128:(j + 1) * 128].T, -1e30)
         for j in range(4)], axis=0)).astype(np.float32)
    cosT = np.asarray(freqs_cos, np.float32).T   # [64, S]
    sinT = np.asarray(freqs_sin, np.float32).T
    c2 = np.ascontiguousarray(np.concatenate([cosT, cosT], axis=0))
    s2 = np.ascontiguousarray(np.concatenate([-sinT, sinT], axis=0))
    wqkv = np.asarray(wqkv, np.float32)
    wo = np.asarray(wo, np.float32)
    x = np.asarray(x, np.float32)
    woT = np.ascontiguousarray(wo.T).astype(bf)   # [2048 o, 2048 e]

    in_maps = []
    for c in range(8):
        b, r = divmod(c, TP)
        heads = list(range(r * HL, (r + 1) * HL))
        # q/k weights: per (sec, head) block in SBUF layout [128 p=d%128,
        # (dt c)=hd], i.e. transpose of blk[c, dt*128+p]
        rows = []
        for sec in range(2):
            for h in heads:
                blk = wqkv[sec * D + h * HD: sec * D + (h + 1) * HD][perm]
                b3 = blk.reshape(HD, NDT, 128)          # [hd, dt, p]
                rows.append(np.transpose(b3, (2, 1, 0)).reshape(128, -1))
        wqk = np.ascontiguousarray(np.concatenate(rows, axis=0)).astype(bf)
        wv = np.ascontiguousarray(np.concatenate(
            [wqkv[2 * D + h * HD: 2 * D + (h + 1) * HD].T for h in heads],
            axis=1)).astype(bf)                          # [2048, 512]
        xt = np.ascontiguousarray(x[b].T).astype(bf)
        in_maps.append({"xt": xt, "wqk": wqk, "wv": wv, "c2": c2, "s2": s2,
                        "maskT": maskT, "woT": woT})
    return in_maps


def kernel(x, freqs_cos, freqs_sin, mask, wqkv, wo, input_pos=None,
           _want_res=False, _trace=False, _tmpdir=None):
    from concourse.bass_utils import run_bass_kernel_spmd

    if "nc" not in _cache:
        _cache["nc"] = _build_graph()
    nc = _cache["nc"]

    in_maps = _prep_inputs(x, freqs_cos, freqs_sin, mask, wqkv, wo)
    kw = {}
    if _trace:
        kw = dict(trace=True, tmpdir=_tmpdir)
    res = run_bass_kernel_spmd(nc, in_maps, list(range(8)), **kw)

    y = np.empty((B, S, D), np.float32)
    for c in range(8):
        b, r = divmod(c, TP)
        oc = np.asarray(res.results[c]["out"], np.float32)
        for qc in range(NQC):
            qt = 4 * qc + r
            y[b, qt * 128:(qt + 1) * 128, :] = oc[qc * 128:(qc + 1) * 128]
    if _want_res:
        return y, res
    return y


# revision 31
# speedup vs baseline: 1.0166x; 1.0166x over previous
"""Trainium2 Bass kernel for multi-head attention (B=2, S=2048, D=2048, 16 heads).

Sharding: 8 cores = 2 batch groups (data parallel) x 4 tensor-parallel ranks.
Each core computes QKV + attention for its 4 heads over its batch element.
Per 512-row query chunk the cores exchange their (normalized) attention
outputs O^T with an 8-way AllToAll (one per head-pair half), then each core
contracts the full 2048-dim O rows of the query subtile it owns against the
full wo^T.  This moves ~2x fewer collective bytes than reduce-scattering
partial Y and moves the exchange before the out-projection, shrinking the
kernel tail.  The A2A must span all 8 cores (mesh needs >4), so each core
writes its blocks into both batch-groups' destination slots and picks the
correct source half with rank-conditional DMAs (cc_rank).

Layout:
- All device matmuls contract over the partition dim.  Host pre-transposes:
  xt = x^T, per-head q/k weights as [d, hd] blocks, wv as [d, vcols],
  woT = wo^T.
- Q/K are produced in [hd, s] layout (RoPE pairs permuted even|odd so the
  rotation acts on partition halves); V is produced directly in natural
  [s, hd] layout (stationary = xt tile), so no PE transposes anywhere.
- Scores are computed transposed [k, q]:  exp tiles feed PV directly
  (O^T accumulates in PSUM) and the softmax denominator comes from a
  ones-vector matmul; normalization multiplies O^T by a partition-broadcast
  reciprocal.  Softmax scale is folded into the Exp activation.
"""

import sys
import numpy as np
import ml_dtypes

sys.path.insert(0, "/opt/trn_rl_repo")

B, S, D = 2, 2048, 2048
NH, HD = 16, 128
TP = 4            # tensor-parallel ranks per batch group
HL = NH // TP     # heads per core = 4
NDT = D // 128    # 16 d-tiles
NSC = 4           # 512-col s chunks
NQT = S // 128    # 16
NQC = 4           # 512-row query chunks
SM_SCALE = float(HD) ** -0.5
GROUPS8 = [[0, 1, 2, 3, 4, 5, 6, 7]]
CHUNK_ORDER = [2, 3, 1, 0]

_cache = {}


def _build_graph():
    import concourse.mybir as mybir
    import concourse.tile as tile
    from concourse import bacc

    f32 = mybir.dt.float32
    bf16 = mybir.dt.bfloat16
    AF = mybir.ActivationFunctionType
    OP = mybir.AluOpType

    nc = bacc.Bacc("TRN2", target_bir_lowering=False, debug=False, num_devices=8)

    xt_ext = nc.declare_dram_parameter("xt", [D, S], bf16, isOutput=False)
    wqk_ext = nc.declare_dram_parameter("wqk", [2 * HL * 128, NDT * 128], bf16,
                                        isOutput=False)
    wv_ext = nc.declare_dram_parameter("wv", [D, HL * HD], bf16, isOutput=False)
    c2_ext = nc.declare_dram_parameter("c2", [128, S], f32, isOutput=False)
    s2_ext = nc.declare_dram_parameter("s2", [128, S], f32, isOutput=False)
    maskT_ext = nc.declare_dram_parameter("maskT", [512, 512], f32, isOutput=False)
    woT_ext = nc.declare_dram_parameter("woT", [D, D], bf16, isOutput=False)
    out_ext = nc.declare_dram_parameter("out", [NQC * 128, D], bf16, isOutput=True)

    with tile.TileContext(nc) as tc:
        with tc.tile_pool(name="pers", bufs=1) as pers, \
             tc.tile_pool(name="dram", bufs=1, space="DRAM") as dram:
            qk_bf = [pers.tile([128, S], bf16, tag=f"qk{i}", name=f"qk{i}")
                     for i in range(2 * HL)]            # 0-3 q heads, 4-7 k heads
            v_bf = [pers.tile([128, HL * HD], bf16, tag=f"v{t}", name=f"v{t}")
                    for t in range(NQT)]                # [s-tile, 4*hd]
            ones_col = pers.tile([128, 1], bf16, tag="ones", name="ones")
            nc.vector.memset(ones_col[:], 1.0)

            # ---------------- Phase A: QKV projection + RoPE ----------------
            with tc.tile_pool(name="pha", bufs=1) as pha, \
                 tc.tile_pool(name="rope", bufs=2) as ropep, \
                 tc.tile_pool(name="psA", bufs=3, space="PSUM") as psA:
                wq_sb = [pha.tile([128, NDT * 128], bf16, tag=f"wq{et}",
                                  name=f"wq{et}") for et in range(2 * HL)]
                wv_sb = [pha.tile([128, HL * HD], bf16, tag=f"wv{dt}",
                                  name=f"wv{dt}") for dt in range(NDT)]
                xt_sb = [pha.tile([128, S], bf16, tag=f"xt{dt}", name=f"xt{dt}")
                         for dt in range(NDT)]
                c2_sb = pha.tile([128, S], f32, tag="c2", name="c2")
                s2_sb = pha.tile([128, S], f32, tag="s2", name="s2")

                def dma_xt_chunk(sc):
                    for dt in range(NDT):
                        nc.sync.dma_start(
                            out=xt_sb[dt][:, sc * 512:(sc + 1) * 512],
                            in_=xt_ext[dt * 128:(dt + 1) * 128,
                                       sc * 512:(sc + 1) * 512])

                def dma_tab_chunk(sc):
                    cl = slice(sc * 512, (sc + 1) * 512)
                    nc.sync.dma_start(out=c2_sb[:, cl], in_=c2_ext[:, cl])
                    nc.sync.dma_start(out=s2_sb[:, cl], in_=s2_ext[:, cl])

                # DMA issue order tuned so compute starts after ~2.5 MB
                nc.sync.dma_start(out=wq_sb[0][:], in_=wqk_ext[0:128, :])
                dma_xt_chunk(0)
                for et in range(1, 4):
                    nc.sync.dma_start(out=wq_sb[et][:],
                                      in_=wqk_ext[et * 128:(et + 1) * 128, :])
                dma_tab_chunk(0)
                for et in range(4, 2 * HL):
                    nc.sync.dma_start(out=wq_sb[et][:],
                                      in_=wqk_ext[et * 128:(et + 1) * 128, :])
                for dt in range(NDT):
                    nc.sync.dma_start(out=wv_sb[dt][:],
                                      in_=wv_ext[dt * 128:(dt + 1) * 128, :])
                for sc in range(1, NSC):
                    dma_xt_chunk(sc)
                    dma_tab_chunk(sc)

                for sc in range(NSC):
                    cl = slice(sc * 512, (sc + 1) * 512)
                    for et in range(2 * HL):
                        ps = psA.tile([128, 512], f32, tag="psA",
                                      name=f"psA_{sc}_{et}")
                        for dt in range(NDT):
                            nc.tensor.matmul(
                                ps[:], wq_sb[et][:, dt * 128:(dt + 1) * 128],
                                xt_sb[dt][:, cl],
                                start=(dt == 0), stop=(dt == NDT - 1))
                        # u = [r*c; i*c]; w = [-i*s; r*s] (s2n = [-sin; sin],
                        # cross-partition reads stay on the PSUM operand);
                        # qk = u + w = [r*c - i*s; i*c + r*s]
                        u = ropep.tile([128, 512], f32, tag="t1",
                                       name=f"t1_{sc}_{et}")
                        w = ropep.tile([128, 512], f32, tag="t2",
                                       name=f"t2_{sc}_{et}")
                        nc.vector.tensor_tensor(out=u[:], in0=ps[:],
                                                in1=c2_sb[:, cl], op=OP.mult)
                        nc.vector.tensor_tensor(out=w[0:64, :],
                                                in0=ps[64:128, :],
                                                in1=s2_sb[0:64, cl],
                                                op=OP.mult)
                        nc.vector.tensor_tensor(out=w[64:128, :],
                                                in0=ps[0:64, :],
                                                in1=s2_sb[64:128, cl],
                                                op=OP.mult)
                        nc.vector.tensor_tensor(out=qk_bf[et][:, cl],
                                                in0=u[:], in1=w[:], op=OP.add)
                    for stl in range(4):
                        st = sc * 4 + stl
                        psv = psA.tile([128, 512], f32, tag="psA",
                                       name=f"psV_{st}")
                        for dt in range(NDT):
                            nc.tensor.matmul(
                                psv[:], xt_sb[dt][:, st * 128:(st + 1) * 128],
                                wv_sb[dt][:],
                                start=(dt == 0), stop=(dt == NDT - 1))
                        nc.scalar.copy(v_bf[st][:], psv[:])

            # -------- Phase B: attention + AllToAll + local out-proj --------
            with tc.tile_pool(name="phb", bufs=1) as phb, \
                 tc.tile_pool(name="att", bufs=4) as attp, \
                 tc.tile_pool(name="psS", bufs=2, space="PSUM") as psS, \
                 tc.tile_pool(name="psPV", bufs=2, space="PSUM") as psPV, \
                 tc.tile_pool(name="psD", bufs=2, space="PSUM") as psD, \
                 tc.tile_pool(name="psPR", bufs=2, space="PSUM") as psPR:
                maskT_sb = [phb.tile([128, 512], f32, tag=f"mk{j}",
                                     name=f"mk{j}") for j in range(4)]
                woT_sb = [phb.tile([128, D], bf16, tag=f"wo{k}", name=f"wo{k}")
                          for k in range(NDT)]
                for j in range(4):
                    nc.sync.dma_start(out=maskT_sb[j][:],
                                      in_=maskT_ext[j * 128:(j + 1) * 128, :])
                for k in range(NDT):
                    nc.sync.dma_start(out=woT_sb[k][:],
                                      in_=woT_ext[k * 128:(k + 1) * 128, :])

                rank = nc.sync.cc_rank(replica_groups=GROUPS8)
                is_b0 = rank < 4
                is_b1 = rank >= 4
                a2a_out = {}

                def attention(qc, hp):
                    qcl = slice(qc * 512, (qc + 1) * 512)
                    nkt = qc * 4 + 4
                    o2p = attp.tile([128, 1024], bf16, tag="o2p",
                                    name=f"o2p_{qc}_{hp}", bufs=3)
                    ps_pv = [psPV.tile([128, 512], f32, tag="pv",
                                       name=f"pv_{qc}_{hp}_{i}")
                             for i in range(2)]
                    ps_d = [psD.tile([1, 512], f32, tag="d",
                                     name=f"d_{qc}_{hp}_{i}") for i in range(2)]
                    for kt in range(nkt):
                        for i in range(2):
                            h = 2 * hp + i
                            ps_s = psS.tile([128, 512], f32, tag="s",
                                            name=f"s_{qc}_{hp}_{kt}_{i}")
                            nc.tensor.matmul(
                                ps_s[:], qk_bf[HL + h][:, kt * 128:(kt + 1) * 128],
                                qk_bf[h][:, qcl], start=True, stop=True)
                            if kt >= qc * 4:
                                nc.vector.tensor_tensor(
                                    out=ps_s[:], in0=ps_s[:],
                                    in1=maskT_sb[kt - qc * 4][:], op=OP.add)
                            e_sb = attp.tile([128, 512], bf16, tag="e",
                                             name=f"e_{qc}_{hp}_{kt}_{i}",
                                             bufs=4)
                            nc.scalar.activation(e_sb[:], ps_s[:], AF.Exp,
                                                 scale=SM_SCALE)
                            nc.tensor.matmul(ps_d[i][:], ones_col[:], e_sb[:],
                                             start=(kt == 0),
                                             stop=(kt == nkt - 1))
                            nc.tensor.matmul(
                                ps_pv[i][:], v_bf[kt][:, h * 128:(h + 1) * 128],
                                e_sb[:], start=(kt == 0), stop=(kt == nkt - 1))
                    for i in range(2):
                        d_sb = attp.tile([1, 512], f32, tag="dsb",
                                         name=f"dsb_{qc}_{hp}_{i}")
                        nc.scalar.copy(d_sb[:], ps_d[i][:])
                        db = attp.tile([128, 512], f32, tag="db",
                                       name=f"db_{qc}_{hp}_{i}", bufs=2)
                        nc.gpsimd.partition_broadcast(db[:], d_sb[:])
                        rb = attp.tile([128, 512], f32, tag="rb",
                                       name=f"rb_{qc}_{hp}_{i}", bufs=2)
                        nc.vector.reciprocal_approx_fast(rb[:], db[:])
                        # o2p column layout is g-major (g = r*2 + i: dest
                        # rank r, pair member i) so the staging DMA is 3-dim
                        nc.vector.tensor_tensor(
                            out=o2p[:].rearrange("p (r i f) -> p i r f",
                                                 i=2, f=128)[:, i],
                            in0=ps_pv[i][:].rearrange("p (r f) -> p r f",
                                                      f=128),
                            in1=rb[:].rearrange("p (r f) -> p r f", f=128),
                            op=OP.mult)
                    # stage O^T blocks to DRAM grouped by destination core
                    # (same block for both batch-groups' slots) and exchange
                    a_in = dram.tile([2048, 128], bf16, tag=f"ain_{qc}_{hp}",
                                     name=f"ain_{qc}_{hp}")
                    src = o2p[:].rearrange("p (g f) -> p g f", f=128)
                    for half in range(2):
                        dst = a_in[half * 1024:(half + 1) * 1024, :].rearrange(
                            "(g p) f -> p g f", p=128)
                        nc.sync.dma_start(out=dst, in_=src)
                    a_out = dram.tile([2048, 128], bf16, tag=f"aout_{qc}_{hp}",
                                      name=f"aout_{qc}_{hp}")
                    nc.gpsimd.collective_compute(
                        "AllToAll", OP.bypass, replica_groups=GROUPS8,
                        ins=[a_in[:].opt()], outs=[a_out[:].opt()])
                    a2a_out[(qc, hp)] = a_out

                def outproj(qc):
                    # lhs_hp block g = r*2 + i holds global ocol block (head)
                    # k = r*4 + 2*hp + i.  Separate tiles per hp so the
                    # conservative whole-tile deps of the conditional DMAs
                    # don't make the hp0 matmuls wait on the hp1 AllToAll.
                    lhs = [attp.tile([128, 1024], bf16, tag=f"lhs{hp}",
                                     name=f"lhs_{qc}_{hp}", bufs=2)
                           for hp in range(2)]
                    for hp in range(2):
                        a_out = a2a_out[(qc, hp)]
                        dst = lhs[hp][:].rearrange("p (g f) -> p g f", f=128)
                        for b, cond in ((0, is_b0), (1, is_b1)):
                            src = a_out[b * 1024:(b + 1) * 1024, :].rearrange(
                                "(g p) f -> p g f", p=128)
                            nc.sync.dma_start(out=dst, in_=src, cond=cond)
                    # ec columns run in pairs, hp0 blocks first across the
                    # pair, so the first half of the contraction can run
                    # while the hp1 AllToAll is still in flight
                    for eca, ecb in ((0, 1), (2, 3)):
                        pss = {ec: psPR.tile([128, 512], f32, tag="pr",
                                             name=f"pr_{qc}_{ec}")
                               for ec in (eca, ecb)}
                        for hp in range(2):
                            for ec in (eca, ecb):
                                for n in range(8):
                                    r, i = divmod(n, 2)
                                    k = r * HL + 2 * hp + i
                                    nc.tensor.matmul(
                                        pss[ec][:],
                                        lhs[hp][:, n * 128:(n + 1) * 128],
                                        woT_sb[k][:, ec * 512:(ec + 1) * 512],
                                        start=(hp == 0 and n == 0),
                                        stop=(hp == 1 and n == 7))
                        for ec in (eca, ecb):
                            fin = attp.tile([128, 512], bf16, tag="fin",
                                            name=f"fin_{qc}_{ec}", bufs=4)
                            nc.scalar.copy(fin[:], pss[ec][:])
                            nc.sync.dma_start(
                                out=out_ext[qc * 128:(qc + 1) * 128,
                                            ec * 512:(ec + 1) * 512],
                                in_=fin[:])

                prev = None
                for qc in CHUNK_ORDER:
                    attention(qc, 0)
                    if prev is not None:
                        outproj(prev)
                    attention(qc, 1)
                    prev = qc
                outproj(prev)
    nc.finalize()
    return nc


def _prep_inputs(x, freqs_cos, freqs_sin, mask, wqkv, wo):
    bf = ml_dtypes.bfloat16
    perm = np.concatenate([np.arange(0, HD, 2), np.arange(1, HD, 2)])
    mask2d = np.asarray(mask, np.float32).reshape(S, S)
    maskT = np.ascontiguousarray(np.concatenate(
        [np.maximum(mask2d[0:512, j * 128:(j + 1) * 128].T, -1e30)
         for j in range(4)], axis=0)).astype(np.float32)
    cosT = np.asarray(freqs_cos, np.float32).T   # [64, S]
    sinT = np.asarray(freqs_sin, np.float32).T
    c2 = np.ascontiguousarray(np.concatenate([cosT, cosT], axis=0))
    s2 = np.ascontiguousarray(np.concatenate([-sinT, sinT], axis=0))
    wqkv = np.asarray(wqkv, np.float32)
    wo = np.asarray(wo, np.float32)
    x = np.asarray(x, np.float32)
    woT = np.ascontiguousarray(wo.T).astype(bf)   # [2048 o, 2048 e]

    in_maps = []
    for c in range(8):
        b, r = divmod(c, TP)
        heads = list(range(r * HL, (r + 1) * HL))
        # q/k weights: per (sec, head) block in SBUF layout [128 p=d%128,
        # (dt c)=hd], i.e. transpose of blk[c, dt*128+p]
        rows = []
        for sec in range(2):
            for h in heads:
                blk = wqkv[sec * D + h * HD: sec * D + (h + 1) * HD][perm]
                b3 = blk.reshape(HD, NDT, 128)          # [hd, dt, p]
                rows.append(np.transpose(b3, (2, 1, 0)).reshape(128, -1))
        wqk = np.ascontiguousarray(np.concatenate(rows, axis=0)).astype(bf)
        wv = np.ascontiguousarray(np.concatenate(
            [wqkv[2 * D + h * HD: 2 * D + (h + 1) * HD].T for h in heads],
            axis=1)).astype(bf)                          # [2048, 512]
        xt = np.ascontiguousarray(x[b].T).astype(bf)
        in_maps.append({"xt": xt, "wqk": wqk, "wv": wv, "c2": c2, "s2": s2,
                        "maskT": maskT, "woT": woT})
    return in_maps


def kernel(x, freqs_cos, freqs_sin, mask, wqkv, wo, input_pos=None,
           _want_res=False, _trace=False, _tmpdir=None):
    from concourse.bass_utils import run_bass_kernel_spmd

    if "nc" not in _cache:
        _cache["nc"] = _build_graph()
    nc = _cache["nc"]

    in_maps = _prep_inputs(x, freqs_cos, freqs_sin, mask, wqkv, wo)
    kw = {}
    if _trace:
        kw = dict(trace=True, tmpdir=_tmpdir)
    res = run_bass_kernel_spmd(nc, in_maps, list(range(8)), **kw)

    y = np.empty((B, S, D), np.float32)
    for c in range(8):
        b, r = divmod(c, TP)
        oc = np.asarray(res.results[c]["out"], np.float32)
        for qc in range(NQC):
            qt = 4 * qc + r
            y[b, qt * 128:(qt + 1) * 128, :] = oc[qc * 128:(qc + 1) * 128]
    if _want_res:
        return y, res
    return y
